# revision 28
# baseline (speedup 1.0000x reference)
"""Per-node neighbor attention (B=1, N=50000, K=32, D=128) on 8 TRN2 NeuronCores.

out[n] = h[n] + sum_k softmax_k(h[n]·nb[n,k]/sqrt(D)) * nb[n,k]

Sharding: node-parallel, N split evenly across 8 cores (6250 nodes/core);
no cross-core communication.

Per-core pipeline (nodes-on-partitions, 128-node sub-tiles, one SWDGE
cast-DMA per sub-tile prefetched 12 sub-tiles ahead — fine granularity
keeps the arrival quantum at 2.1MB so compute never waits long, and the
deep queue keeps all 16 SDMA engines saturated):
  phase A(t): tmp = nb*h (h broadcast over k) on VectorE (bf16 2x);
    scores: tmp streamed through TensorE with an identity stationary
    (8 f=512 chunks accumulated in PSUM [128,32,16]) + one VectorE
    reduce; p = exp(scores/sqrt(D)) broadcast over d written by ScalarE
    IN PLACE over tmp (no max subtraction: randn inputs keep scores
    ~N(0,1)); the activation's accum_out side-output gives D*sum_k p,
    whose reciprocal feeds the output (the stray D cancels against the
    D*I stationary used in phase B); normalization deferred to the end.
  phase B(t-lag): tmp *= nb in place on VectorE; agg over k via
    TensorE D*I-stationary chunks into PSUM [128,4,128] + a strided
    VectorE reduce; out = h + agg*recip fused on VectorE.
All of h is cast-DMA'd upfront to one persistent bf16 tile on the SAME
gpsimd SWDGE queue as the neighbor loads (head before the first
neighbor sub-tile, tail after the seventh) — h on the HW queue starves
behind the neighbor burst, and per-macro h loads stall mid-pipeline.
bf16 h is used for both the multiply and the final add (~1e-2 worst
abs error, inside the 2e-2 budget).  The last sub-tile's h column is
memset first: raw SBUF garbage there can be NaN, which would spread to
every partition through the identity matmul (NaN*0=NaN in the PE sum).
A burst of accumulating identity matmuls right after the identity
loads warms the PE HAM clock gate (1.2->2.4GHz) before sub-tile 0.
GpSimd runs no compute (it would lock VectorE out of its dual-port 2x
mode) — it only issues the SWDGE cast-DMAs (f32 HBM -> bf16 SBUF).
"""

import os

os.environ.setdefault("NEURON_RT_RESET_CORES", "1")

import numpy as np
import ml_dtypes

import concourse.bass as bass
import concourse.bacc as bacc
import concourse.tile as tile
from concourse import mybir
from concourse.bass_utils import run_bass_kernel_spmd

B, N, K, D = 1, 50000, 32, 128
NCORES = 8
NPC = N // NCORES          # 6250 nodes per core
P = 128                    # nodes per sub-tile (partitions)
N_FULL_SUB = NPC // P      # 48 full sub-tiles
REM = NPC - N_FULL_SUB * P  # 106 remainder nodes
SCALE = float(1.0 / np.sqrt(np.float32(D)))
PREFETCH = 12              # sub-tiles of neighbor-DMA lookahead
LAG = 2                    # sub-tiles between phase A and phase B
HEAD = 6                   # sub-tiles of h loaded before the first nb DMA
WARMUP_MM = 40             # identity matmuls to warm the PE clock gate

bf16 = mybir.dt.bfloat16
f32 = mybir.dt.float32
Alu = mybir.AluOpType
Act = mybir.ActivationFunctionType


def _ap(ap: bass.AP, dims) -> bass.AP:
    return bass.AP(tensor=ap.tensor, offset=ap.offset, ap=dims)


def _build_module():
    nc = bacc.Bacc("TRN2", target_bir_lowering=False, debug=False, num_devices=NCORES)
    h_d = nc.dram_tensor("h", [NPC, D], f32, kind="ExternalInput").ap()
    nb_d = nc.dram_tensor("nb", [NPC, K * D], f32, kind="ExternalInput").ap()
    id_d = nc.dram_tensor("iden", [P, P], bf16, kind="ExternalInput").ap()
    idD_d = nc.dram_tensor("idenD", [P, P], bf16, kind="ExternalInput").ap()
    out_d = nc.dram_tensor("out", [NPC, D], f32, kind="ExternalOutput").ap()

    n_sub = N_FULL_SUB + (1 if REM else 0)          # 49

    with tile.TileContext(nc) as tc:
        with (
            tc.tile_pool(name="pers", bufs=1) as pers,
            tc.tile_pool(name="nbp", bufs=14) as nbp,
            tc.tile_pool(name="tmpp", bufs=4) as tmpp,
            tc.tile_pool(name="small", bufs=8) as small,
            tc.tile_pool(name="outp", bufs=4) as outp,
            tc.tile_pool(name="psum", bufs=4, space="PSUM") as psum,
        ):
            id16 = pers.tile([P, P], bf16)
            nc.sync.dma_start(id16, id_d)
            idD = pers.tile([P, P], bf16)
            nc.sync.dma_start(idD, idD_d)

            # PE clock-gate warmup: ~4.5us of accumulating identity matmuls
            # while the first neighbor tiles stream in.
            ps_w = psum.tile([P, K, 16], f32, tag="ps1")
            for i in range(WARMUP_MM):
                nc.tensor.matmul(
                    ps_w[:, :8, :], lhsT=id16, rhs=id16,
                    start=(i == 0), stop=(i == WARMUP_MM - 1),
                )

            h16all = pers.tile([P, n_sub, D], bf16)
            if REM:
                # zero the whole remainder column first (engine APs cannot
                # start at partition 106); the partial h DMA then overwrites
                # rows :REM.  Without this, raw SBUF garbage (possibly NaN)
                # in rows REM: would spread to all partitions through the
                # identity matmul (NaN*0=NaN in the PE sum).
                nc.vector.memset(h16all[:, N_FULL_SUB, :], 0.0)
            nc.gpsimd.dma_start(
                out=h16all[:, :HEAD, :],
                in_=h_d[: HEAD * P].rearrange("(t p) d -> p t d", p=P),
            )

            sub_tiles = {}
            sub_state = {}

            def emit_dma(t):
                lo = t * P
                rows = min(P, NPC - lo)
                nb16 = nbp.tile([P, K, D], bf16, tag="nb16")
                nc.gpsimd.dma_start(
                    out=nb16[:rows, :, :],
                    in_=nb_d[lo : lo + rows].rearrange("p (k d) -> p k d", k=K),
                )
                sub_tiles[t] = nb16

            def emit_h_tail():
                nc.gpsimd.dma_start(
                    out=h16all[:, HEAD:N_FULL_SUB, :],
                    in_=h_d[HEAD * P : N_FULL_SUB * P].rearrange(
                        "(t p) d -> p t d", p=P
                    ),
                )
                if REM:
                    nc.gpsimd.dma_start(
                        out=h16all[:REM, N_FULL_SUB, :], in_=h_d[N_FULL_SUB * P :]
                    )

            def phase_a(t):
                nbt = sub_tiles[t][:]

                tmp = tmpp.tile([P, K, D], bf16, tag="tmp")
                h16s = h16all[:, t, :]
                nc.vector.tensor_tensor(
                    out=tmp, in0=nbt,
                    in1=_ap(h16s, [h16s.ap[0], [0, K], h16s.ap[1]]),
                    op=Alu.mult,
                )

                ps1 = psum.tile([P, K, 16], f32, tag="ps1")
                for c in range(8):
                    nc.tensor.matmul(
                        ps1, lhsT=id16, rhs=tmp[:, :, 16 * c : 16 * c + 16],
                        start=(c == 0), stop=(c == 7),
                    )
                scores = small.tile([P, K], f32, tag="scores")
                nc.vector.tensor_reduce(
                    out=scores, in_=ps1, axis=mybir.AxisListType.X, op=Alu.add
                )

                # p = exp(scores*SCALE) broadcast over d, in place over tmp
                # (ScalarE); accum_out = sum over (k,d) = D * sum_k p.
                # recip = 1/(D*sum_k p); the stray D cancels against the
                # D*I stationary of the phase-B matmuls.
                sumexp = small.tile([P, 1], f32, tag="sumexp")
                nc.scalar.activation(
                    out=tmp,
                    in_=_ap(scores[:], [*scores[:].ap, [0, D]]),
                    func=Act.Exp,
                    bias=0.0, scale=SCALE,
                    accum_out=sumexp,
                )
                recip = small.tile([P, 1], f32, tag="recip")
                nc.vector.reciprocal(recip, sumexp)
                sub_state[t] = (nbt, tmp, h16all[:, t, :], recip)

            def phase_b(t):
                nbt, tmp, h16s, recip = sub_state.pop(t)

                nc.vector.tensor_tensor(out=tmp, in0=tmp, in1=nbt, op=Alu.mult)

                ps2 = psum.tile([P, 4, D], f32, tag="ps2")
                for c in range(8):
                    nc.tensor.matmul(
                        ps2, lhsT=idD, rhs=tmp[:, 4 * c : 4 * c + 4, :],
                        start=(c == 0), stop=(c == 7),
                    )
                agg = small.tile([P, D], f32, tag="agg")
                nc.vector.tensor_reduce(
                    out=agg,
                    in_=_ap(ps2[:], [ps2[:].ap[0], [1, D], [D, 4]]),
                    axis=mybir.AxisListType.X, op=Alu.add,
                )

                out_t = outp.tile([P, D], f32, tag="out")
                nc.vector.scalar_tensor_tensor(
                    out=out_t, in0=agg, scalar=recip[:], in1=h16s,
                    op0=Alu.mult, op1=Alu.add,
                )
                rows = min(P, NPC - t * P)
                nc.sync.dma_start(out_d[t * P : t * P + rows], out_t[:rows])

            for t in range(min(7, n_sub)):
                emit_dma(t)
            emit_h_tail()
            for t in range(7, min(PREFETCH + 1, n_sub)):
                emit_dma(t)
            for t in range(n_sub + LAG):
                if t < n_sub:
                    phase_a(t)
                    nxt = t + PREFETCH + 1
                    if nxt < n_sub:
                        emit_dma(nxt)
                if t >= LAG:
                    phase_b(t - LAG)

    nc.compile()
    return nc


_NC = None


def _get_nc():
    global _NC
    if _NC is None:
        _NC = _build_module()
    return _NC


def _in_maps(h_n, neighbor):
    h = np.asarray(h_n, dtype=np.float32).reshape(N, D)
    nb = np.asarray(neighbor, dtype=np.float32).reshape(N, K * D)
    iden = np.eye(P, dtype=ml_dtypes.bfloat16)
    idenD = (np.eye(P, dtype=np.float32) * D).astype(ml_dtypes.bfloat16)
    in_maps = []
    for c in range(NCORES):
        lo, hi = c * NPC, (c + 1) * NPC
        in_maps.append(
            {"h": h[lo:hi], "nb": nb[lo:hi], "iden": iden, "idenD": idenD}
        )
    return in_maps


def kernel(h_n, neighbor):
    in_maps = _in_maps(h_n, neighbor)
    nc = _get_nc()
    res = run_bass_kernel_spmd(nc, in_maps, core_ids=list(range(NCORES)))
    out = np.concatenate([r["out"] for r in res.results], axis=0)
    return out.reshape(B, N, D).astype(np.float32)


# revision 29
# speedup vs baseline: 1.1307x; 1.1307x over previous
"""Per-node neighbor attention (B=1, N=50000, K=32, D=128) on 8 TRN2 NeuronCores.

out[n] = h[n] + sum_k softmax_k(h[n]·nb[n,k]/sqrt(D)) * nb[n,k]

Sharding: node-parallel, N split evenly across 8 cores (6250 nodes/core);
no cross-core communication.

Per-core pipeline (nodes-on-partitions, 128-node sub-tiles, one SWDGE
cast-DMA per sub-tile prefetched 14 sub-tiles ahead — fine granularity
keeps the arrival quantum at 2.1MB so compute never waits long, and the
deep queue keeps all 16 SDMA engines saturated):
  phase A(t): tmp = nb*h (h broadcast over k) on VectorE (bf16 2x);
    scores: tmp streamed through TensorE with an identity stationary
    (8 f=512 chunks accumulated in PSUM [128,32,16]) + one VectorE
    reduce; p = exp(scores/sqrt(D)) broadcast over d written by ScalarE
    IN PLACE over tmp (no max subtraction: randn inputs keep scores
    ~N(0,1)); the activation's accum_out side-output gives D*sum_k p,
    whose reciprocal feeds the output (the stray D cancels against the
    D*I stationary used in phase B); normalization deferred to the end.
  phase B(t-lag): tmp *= nb in place on VectorE; agg over k via
    TensorE D*I-stationary chunks into PSUM [128,4,128] + a strided
    VectorE reduce; out = h + agg*recip fused on VectorE.
All of h is cast-DMA'd upfront to one persistent bf16 tile on the SAME
gpsimd SWDGE queue as the neighbor loads (head before the first
neighbor sub-tile, tail after the seventh) — h on the HW queue starves
behind the neighbor burst, and per-macro h loads stall mid-pipeline.
bf16 h is used for both the multiply and the final add (~1e-2 worst
abs error, inside the 2e-2 budget).  The last sub-tile's h column is
memset first: raw SBUF garbage there can be NaN, which would spread to
every partition through the identity matmul (NaN*0=NaN in the PE sum).
A burst of accumulating identity matmuls right after the identity
loads warms the PE HAM clock gate (1.2->2.4GHz) before sub-tile 0.
GpSimd runs no compute (it would lock VectorE out of its dual-port 2x
mode) — it only issues the SWDGE cast-DMAs (f32 HBM -> bf16 SBUF).
"""

import os

os.environ.setdefault("NEURON_RT_RESET_CORES", "1")

import numpy as np
import ml_dtypes

import concourse.bass as bass
import concourse.bacc as bacc
import concourse.tile as tile
from concourse import mybir
from concourse.bass_utils import run_bass_kernel_spmd

B, N, K, D = 1, 50000, 32, 128
NCORES = 8
NPC = N // NCORES          # 6250 nodes per core
P = 128                    # nodes per sub-tile (partitions)
N_FULL_SUB = NPC // P      # 48 full sub-tiles
REM = NPC - N_FULL_SUB * P  # 106 remainder nodes
SCALE = float(1.0 / np.sqrt(np.float32(D)))
PREFETCH = 14              # sub-tiles of neighbor-DMA lookahead
LAG = 2                    # sub-tiles between phase A and phase B
HEAD = 6                   # sub-tiles of h loaded before the first nb DMA
WARMUP_MM = 40             # identity matmuls to warm the PE clock gate

bf16 = mybir.dt.bfloat16
f32 = mybir.dt.float32
Alu = mybir.AluOpType
Act = mybir.ActivationFunctionType


def _ap(ap: bass.AP, dims) -> bass.AP:
    return bass.AP(tensor=ap.tensor, offset=ap.offset, ap=dims)


def _build_module():
    nc = bacc.Bacc("TRN2", target_bir_lowering=False, debug=False, num_devices=NCORES)
    h_d = nc.dram_tensor("h", [NPC, D], f32, kind="ExternalInput").ap()
    nb_d = nc.dram_tensor("nb", [NPC, K * D], f32, kind="ExternalInput").ap()
    id_d = nc.dram_tensor("iden", [P, P], bf16, kind="ExternalInput").ap()
    idD_d = nc.dram_tensor("idenD", [P, P], bf16, kind="ExternalInput").ap()
    out_d = nc.dram_tensor("out", [NPC, D], f32, kind="ExternalOutput").ap()

    n_sub = N_FULL_SUB + (1 if REM else 0)          # 49

    with tile.TileContext(nc) as tc:
        with (
            tc.tile_pool(name="pers", bufs=1) as pers,
            tc.tile_pool(name="nbp", bufs=16) as nbp,
            tc.tile_pool(name="tmpp", bufs=4) as tmpp,
            tc.tile_pool(name="small", bufs=8) as small,
            tc.tile_pool(name="outp", bufs=4) as outp,
            tc.tile_pool(name="psum", bufs=4, space="PSUM") as psum,
        ):
            id16 = pers.tile([P, P], bf16)
            nc.sync.dma_start(id16, id_d)
            idD = pers.tile([P, P], bf16)
            nc.sync.dma_start(idD, idD_d)

            # PE clock-gate warmup: ~4.5us of accumulating identity matmuls
            # while the first neighbor tiles stream in.
            ps_w = psum.tile([P, K, 16], f32, tag="ps1")
            for i in range(WARMUP_MM):
                nc.tensor.matmul(
                    ps_w[:, :8, :], lhsT=id16, rhs=id16,
                    start=(i == 0), stop=(i == WARMUP_MM - 1),
                )

            h16all = pers.tile([P, n_sub, D], bf16)
            if REM:
                # zero the whole remainder column first (engine APs cannot
                # start at partition 106); the partial h DMA then overwrites
                # rows :REM.  Without this, raw SBUF garbage (possibly NaN)
                # in rows REM: would spread to all partitions through the
                # identity matmul (NaN*0=NaN in the PE sum).
                nc.vector.memset(h16all[:, N_FULL_SUB, :], 0.0)
            nc.gpsimd.dma_start(
                out=h16all[:, :HEAD, :],
                in_=h_d[: HEAD * P].rearrange("(t p) d -> p t d", p=P),
            )

            sub_tiles = {}
            sub_state = {}

            def emit_dma(t):
                lo = t * P
                rows = min(P, NPC - lo)
                nb16 = nbp.tile([P, K, D], bf16, tag="nb16")
                nc.gpsimd.dma_start(
                    out=nb16[:rows, :, :],
                    in_=nb_d[lo : lo + rows].rearrange("p (k d) -> p k d", k=K),
                )
                sub_tiles[t] = nb16

            def emit_h_tail():
                nc.gpsimd.dma_start(
                    out=h16all[:, HEAD:N_FULL_SUB, :],
                    in_=h_d[HEAD * P : N_FULL_SUB * P].rearrange(
                        "(t p) d -> p t d", p=P
                    ),
                )
                if REM:
                    nc.gpsimd.dma_start(
                        out=h16all[:REM, N_FULL_SUB, :], in_=h_d[N_FULL_SUB * P :]
                    )

            def phase_a(t):
                nbt = sub_tiles[t][:]

                tmp = tmpp.tile([P, K, D], bf16, tag="tmp")
                h16s = h16all[:, t, :]
                nc.vector.tensor_tensor(
                    out=tmp, in0=nbt,
                    in1=_ap(h16s, [h16s.ap[0], [0, K], h16s.ap[1]]),
                    op=Alu.mult,
                )

                ps1 = psum.tile([P, K, 16], f32, tag="ps1")
                for c in range(8):
                    nc.tensor.matmul(
                        ps1, lhsT=id16, rhs=tmp[:, :, 16 * c : 16 * c + 16],
                        start=(c == 0), stop=(c == 7),
                    )
                scores = small.tile([P, K], f32, tag="scores")
                nc.vector.tensor_reduce(
                    out=scores, in_=ps1, axis=mybir.AxisListType.X, op=Alu.add
                )

                # p = exp(scores*SCALE) broadcast over d, in place over tmp
                # (ScalarE); accum_out = sum over (k,d) = D * sum_k p.
                # recip = 1/(D*sum_k p); the stray D cancels against the
                # D*I stationary of the phase-B matmuls.
                sumexp = small.tile([P, 1], f32, tag="sumexp")
                nc.scalar.activation(
                    out=tmp,
                    in_=_ap(scores[:], [*scores[:].ap, [0, D]]),
                    func=Act.Exp,
                    bias=0.0, scale=SCALE,
                    accum_out=sumexp,
                )
                recip = small.tile([P, 1], f32, tag="recip")
                nc.vector.reciprocal(recip, sumexp)
                sub_state[t] = (nbt, tmp, h16all[:, t, :], recip)

            def phase_b(t):
                nbt, tmp, h16s, recip = sub_state.pop(t)

                nc.vector.tensor_tensor(out=tmp, in0=tmp, in1=nbt, op=Alu.mult)

                ps2 = psum.tile([P, 4, D], f32, tag="ps2")
                for c in range(8):
                    nc.tensor.matmul(
                        ps2, lhsT=idD, rhs=tmp[:, 4 * c : 4 * c + 4, :],
                        start=(c == 0), stop=(c == 7),
                    )
                agg = small.tile([P, D], f32, tag="agg")
                nc.vector.tensor_reduce(
                    out=agg,
                    in_=_ap(ps2[:], [ps2[:].ap[0], [1, D], [D, 4]]),
                    axis=mybir.AxisListType.X, op=Alu.add,
                )

                out_t = outp.tile([P, D], f32, tag="out")
                nc.vector.scalar_tensor_tensor(
                    out=out_t, in0=agg, scalar=recip[:], in1=h16s,
                    op0=Alu.mult, op1=Alu.add,
                )
                rows = min(P, NPC - t * P)
                nc.sync.dma_start(out_d[t * P : t * P + rows], out_t[:rows])

            for t in range(min(7, n_sub)):
                emit_dma(t)
            emit_h_tail()
            for t in range(7, min(PREFETCH + 1, n_sub)):
                emit_dma(t)
            for t in range(n_sub + LAG):
                if t < n_sub:
                    phase_a(t)
                    nxt = t + PREFETCH + 1
                    if nxt < n_sub:
                        emit_dma(nxt)
                if t >= LAG:
                    phase_b(t - LAG)

    nc.compile()
    return nc


_NC = None


def _get_nc():
    global _NC
    if _NC is None:
        _NC = _build_module()
    return _NC


def _in_maps(h_n, neighbor):
    h = np.asarray(h_n, dtype=np.float32).reshape(N, D)
    nb = np.asarray(neighbor, dtype=np.float32).reshape(N, K * D)
    iden = np.eye(P, dtype=ml_dtypes.bfloat16)
    idenD = (np.eye(P, dtype=np.float32) * D).astype(ml_dtypes.bfloat16)
    in_maps = []
    for c in range(NCORES):
        lo, hi = c * NPC, (c + 1) * NPC
        in_maps.append(
            {"h": h[lo:hi], "nb": nb[lo:hi], "iden": iden, "idenD": idenD}
        )
    return in_maps


def kernel(h_n, neighbor):
    in_maps = _in_maps(h_n, neighbor)
    nc = _get_nc()
    res = run_bass_kernel_spmd(nc, in_maps, core_ids=list(range(NCORES)))
    out = np.concatenate([r["out"] for r in res.results], axis=0)
    return out.reshape(B, N, D).astype(np.float32)
